# revision 28
# baseline (speedup 1.0000x reference)
"""DeepseekV2 MLA attention on 8 Trainium2 NeuronCores.

Sharding: token-split A-projections -> AllGather(kv latent, fired early) +
AllGather(q latent) -> head-split (4 heads/core) B-projections + causal
attention -> per-query-block AllGather(attn out) -> D-column-split output
projection. Layouts are d-major (feature dim on the SBUF partition axis).

Pipeline: the kv latent gather fires ~1/3 into phase_a so phase_b (k/v
projections) runs right after phase_a's matmuls; the q-latent gather is
hidden under phase_b; phase_q computes its rope chunks first so the
serialized vector rope chain overlaps the nope matmuls; q stays resident
in SBUF; the first o-projection input block is prefetched during
attention so phase_out starts without a DMA bubble.

Precision: bf16 matmul inputs with fp32 PSUM accumulation throughout;
rmsnorm statistics and softmax run in fp32/f32r.
"""
import math

import numpy as np
import ml_dtypes

import concourse.bass as bass
import concourse.mybir as mybir
from concourse.tile import TileContext
from concourse import bass_utils

# ---------------------------------------------------------------------------
# Walrus workaround: this container's walrus accepts at most ONE sync-wait
# per TPB instruction, but Tile attaches several (tail Drain, LDWEIGHTS...).
# Split: keep the last wait, move the rest onto preceding same-engine NOPs.
# ---------------------------------------------------------------------------
import concourse.tile as _tile_mod

_orig_sched = _tile_mod.TileContext.schedule_and_allocate
_nopctr = [0]


def _split_multiwait(nc):
    for fn in nc.m.functions:
        for blk in fn.blocks:
            insts = blk.instructions
            if not any(
                i.sync_info and i.sync_info.on_wait and len(i.sync_info.on_wait) > 1
                for i in insts
            ):
                continue
            out = []
            for ins in insts:
                si = ins.sync_info
                if si and si.on_wait and len(si.on_wait) > 1:
                    waits = list(si.on_wait)
                    for w in waits[:-1]:
                        _nopctr[0] += 1
                        nop = mybir.InstNoOp(name=f"I-mws-{_nopctr[0]}", ins=[], outs=[])
                        nop.engine = ins.engine
                        nop.sync_info = mybir.SyncInfo(on_wait=[w], on_update=[])
                        out.append(nop)
                    ins.sync_info = mybir.SyncInfo(
                        on_wait=[waits[-1]], on_update=list(si.on_update or [])
                    )
                out.append(ins)
            blk.instructions = out


def _patched_sched(self, *a, **k):
    res = _orig_sched(self, *a, **k)
    _split_multiwait(self.nc)
    return res


if getattr(_tile_mod.TileContext.schedule_and_allocate, "__name__", "") != "_patched_sched":
    _tile_mod.TileContext.schedule_and_allocate = _patched_sched


# ---------------------------------------------------------------------------
T, D, H = 2048, 5120, 32
NOPE, ROPE, QK = 128, 64, 192
KVR, QR, VH = 512, 1536, 128
EPS, THETA = 1e-6, 10000.0
NCORES = 8
HL = H // NCORES          # 4 heads per core
TC = T // NCORES          # 256 tokens per core
LAT = KVR + ROPE          # 576
DCOL = D // NCORES        # 640 output columns per core

F32 = mybir.dt.float32
F32R = mybir.dt.float32r
BF16 = mybir.dt.bfloat16
AF = mybir.ActivationFunctionType
MUL = mybir.AluOpType.mult
ADD = mybir.AluOpType.add
SUB = mybir.AluOpType.subtract

TRACE = [False]          # test.py sets TRACE[0]=True to profile
LAST_RESULT = [None]     # BassKernelResults stashed here for test.py

_cache = {}


def _phase_a(nc, tc, io, consts_t, agkv_in, agkv_out, agql_in, agql_out):
    """Token-split A projections (bf16): q chunks FIRST so AG(q latent) —
    the gather on the critical path to phase_q — fires as early as
    possible; the kv chunks + AG(kv) follow (phase_b consumes them much
    later, so that path has slack)."""
    ones_c, ones_r = consts_t["ones_c"], consts_t["ones_r"]
    cosa_sb, sina_sb, bias_sb = (consts_t["cosa_sb"], consts_t["sina_sb"],
                                 consts_t["bias_sb"])
    with (
        tc.tile_pool(name="a_ht", bufs=1) as ht_pool,
        tc.tile_pool(name="a_w", bufs=3) as a_w,
        tc.tile_pool(name="a_st", bufs=1) as a_st,
        tc.tile_pool(name="a_tmp", bufs=3) as a_tmp,
        tc.tile_pool(name="a_ps", bufs=2, space="PSUM") as a_ps,
        tc.tile_pool(name="a_ss", bufs=1, space="PSUM") as a_ss,
    ):
        ht_sb = ht_pool.tile([128, 40 * TC], BF16, name="ht_sb")
        htv = ht_sb[:].rearrange("p (k t) -> p k t", k=40)
        nc.sync.dma_start(htv, io["hT"][:].rearrange("(k p) t -> p k t", p=128))

        stage = a_st.tile([128, 17 * TC], F32R, name="stage")
        ss_q = a_ss.tile([1, TC], F32, name="ss_q")
        ss_kv = a_ss.tile([1, TC], F32, name="ss_kv")

        def scale_of(ss, nfeat, key):
            """1/sqrt(mean(ss)+eps) broadcast to 128 partitions (f32r)."""
            ms = a_tmp.tile([1, TC], F32R, name=f"ms_{key}", tag="ms")
            nc.vector.tensor_scalar(ms[:], ss[:], 1.0 / nfeat, EPS,
                                    op0=MUL, op1=ADD)
            sq2 = a_tmp.tile([1, TC], F32R, name=f"sqr_{key}", tag="sqr")
            nc.scalar.activation(sq2[:], ms[:], AF.Sqrt)
            rs = a_tmp.tile([1, TC], F32R, name=f"rs_{key}", tag="rs")
            with nc.allow_low_precision(reason="f32r holds full fp32 bits"):
                nc.vector.reciprocal(rs[:], sq2[:])
            bps = a_ps.tile([128, TC], F32, name=f"bps_{key}", tag="bps")
            nc.tensor.matmul(bps[:], ones_r[:1, :], rs[:], start=True, stop=True)
            bc = a_tmp.tile([128, TC], F32R, name=f"bc_{key}", tag=f"bc{key}")
            nc.vector.tensor_copy(bc[:], bps[:])
            return bc

        # ---- q chunks first (stage slots 0..11) ----
        for m in range(12):
            wt = a_w.tile([128, 40 * 128], BF16, name=f"a_wq_{m}", tag="aw")
            wtv = wt[:].rearrange("p (k c) -> p k c", k=40)
            nc.sync.dma_start(
                wtv,
                io["wqa"][:].rearrange("(k p) q -> p k q", p=128)[
                    :, :, m * 128:(m + 1) * 128])
            ps = a_ps.tile([128, TC], F32, name=f"a_psq_{m}", tag="aps")
            for k in range(40):
                nc.tensor.matmul(ps[:], wtv[:, k, :], htv[:, k, :],
                                 start=(k == 0), stop=(k == 39))
            st = stage[:, m * TC:(m + 1) * TC]
            nc.vector.tensor_copy(st, ps[:])
            sq = a_tmp.tile([128, TC], F32R, name=f"sqq_{m}", tag="sq")
            nc.scalar.activation(sq[:], st, AF.Square)
            nc.tensor.matmul(ss_q[:], ones_c, sq[:],
                             start=(m == 0), stop=(m == 11))

        bc_q = scale_of(ss_q, QR, "q")
        for m in range(12):
            st = stage[:, m * TC:(m + 1) * TC]
            sc = a_tmp.tile([128, TC], BF16, name=f"scq_{m}", tag="sc")
            nc.vector.tensor_tensor(sc[:], st, bc_q[:], op=MUL)
            nc.sync.dma_start(agql_in[m * 128:(m + 1) * 128, :], sc[:])
        with nc.named_scope("ag_ql"):
            nc.gpsimd.collective_compute(
                "AllGather", mybir.AluOpType.bypass,
                ins=[agql_in[:]], outs=[agql_out[:]],
                replica_groups=[list(range(NCORES))],
            )

        # ---- kv chunks (stage slots 12..16) ----
        for m in range(5):
            mrows = 64 if m == 4 else 128
            wt = a_w.tile([128, 40 * 128], BF16, name=f"a_wkv_{m}", tag="aw")
            wtv = wt[:].rearrange("p (k c) -> p k c", k=40)
            nc.sync.dma_start(
                wtv[:, :, :mrows],
                io["wkva"][:].rearrange("(k p) q -> p k q", p=128)[
                    :, :, m * 128:m * 128 + mrows])
            ps = a_ps.tile([128, TC], F32, name=f"a_pskv_{m}", tag="aps")
            for k in range(40):
                nc.tensor.matmul(ps[:mrows, :], wtv[:, k, :mrows], htv[:, k, :],
                                 start=(k == 0), stop=(k == 39))
            st = stage[:, (12 + m) * TC:(13 + m) * TC]
            if m < 4:
                nc.vector.tensor_scalar(st, ps[:], bias_sb[:, m:m + 1],
                                        None, op0=ADD)
                sq = a_tmp.tile([128, TC], F32R, name=f"sqkv_{m}", tag="sq")
                nc.scalar.activation(sq[:], st, AF.Square)
                nc.tensor.matmul(ss_kv[:], ones_c, sq[:],
                                 start=(m == 0), stop=(m == 3))
            else:
                nc.vector.tensor_scalar(st[:64, :], ps[:64, :],
                                        bias_sb[:64, 4:5], None, op0=ADD)

        bc_kv = scale_of(ss_kv, KVR, "kv")
        for m in range(4):
            st = stage[:, (12 + m) * TC:(13 + m) * TC]
            sc = a_tmp.tile([128, TC], BF16, name=f"sckv_{m}", tag="sc")
            nc.vector.tensor_tensor(sc[:], st, bc_kv[:], op=MUL)
            nc.sync.dma_start(agkv_in[m * 128:(m + 1) * 128, :], sc[:])

        # k_pe rope (no norm) -> rows 512:576
        st = stage[:, 16 * TC:17 * TC]
        rp = a_tmp.tile([64, TC], BF16, name="rp_kpe")
        t1 = a_tmp.tile([32, TC], F32R, name="rt1", tag="rt1")
        t2 = a_tmp.tile([32, TC], F32R, name="rt2", tag="rt2")
        x1, x2 = st[0:32, :], st[32:64, :]
        nc.vector.tensor_tensor(t1[:], x1, cosa_sb[0:32, :], op=MUL)
        nc.vector.tensor_tensor(t2[:], x2, sina_sb[32:64, :], op=MUL)
        nc.vector.tensor_tensor(rp[0:32, :], t1[:], t2[:], op=SUB)
        nc.vector.tensor_tensor(t1[:], x1, sina_sb[0:32, :], op=MUL)
        nc.vector.tensor_tensor(t2[:], x2, cosa_sb[32:64, :], op=MUL)
        nc.vector.tensor_tensor(rp[32:64, :], t1[:], t2[:], op=ADD)
        nc.sync.dma_start(agkv_in[512:576, :], rp[:])

        with nc.named_scope("ag_kv"):
            nc.gpsimd.collective_compute(
                "AllGather", mybir.AluOpType.bypass,
                ins=[agkv_in[:]], outs=[agkv_out[:]],
                replica_groups=[list(range(NCORES))],
            )


def _phase_b_loads(nc, io, agkvv, kpe_sb, b_kva, b_w):
    """DMA the gathered kv latents + B-projection weights; returns tiles.
    Issued on the SCALAR hw-dma queue so they don't head-of-line block
    the sync queue (they wait on AG(kv), which lands late but with
    slack — phase_b's matmuls run after phase_q's)."""
    wk_sb = b_w.tile([128, 4 * 512], BF16, name="wk_sb", tag="wkw")
    nc.scalar.dma_start(wk_sb[:].rearrange("p (k c) -> p k c", k=4),
                        io["wkvbk"][:].rearrange("(k p) c -> p k c", p=128))
    kva_sb = b_kva.tile([128, 4 * T], BF16, name="kva_sb")
    kvav = kva_sb[:].rearrange("p (k t) -> p k t", k=4)
    for k in range(4):
        nc.scalar.dma_start(
            kvav[:, k, :].rearrange("p (r t) -> p r t", r=NCORES),
            agkvv[k * 128:(k + 1) * 128])
    # k_pe duplicated on both partition halves so attention can feed
    # matmuls whose q slice lives at base partition 0 or 64
    for half in range(2):
        nc.scalar.dma_start(
            kpe_sb[64 * half:64 * half + 64, :].rearrange(
                "p (r t) -> p r t", r=NCORES),
            agkvv[512:576])
    wv_sb = b_w.tile([128, 4 * 512], BF16, name="wv_sb", tag="wvw")
    nc.scalar.dma_start(wv_sb[:].rearrange("p (k c) -> p k c", k=4),
                        io["wkvbv"][:].rearrange("(k p) c -> p k c", p=128))
    return kva_sb, wk_sb, wv_sb


def _phase_b(nc, tc, ktv, vv, kva_sb, wk_sb, wv_sb):
    """Head-split k_nope^T and v projections from the gathered kv latents."""
    kvav = kva_sb[:].rearrange("p (k t) -> p k t", k=4)
    wkv_ = wk_sb[:].rearrange("p (k c) -> p k c", k=4)
    wvv = wv_sb[:].rearrange("p (k c) -> p k c", k=4)
    with tc.tile_pool(name="b_ps", bufs=2, space="PSUM") as b_ps:
        for j in range(HL):
            for qb in range(4):
                ps = b_ps.tile([128, 512], F32, name=f"psk_{j}_{qb}", tag="psk")
                for k in range(4):
                    nc.tensor.matmul(ps[:], wkv_[:, k, j * 128:(j + 1) * 128],
                                     kvav[:, k, qb * 512:(qb + 1) * 512],
                                     start=(k == 0), stop=(k == 3))
                nc.vector.tensor_copy(ktv[:, j, qb * 512:(qb + 1) * 512], ps[:])
        for mt in range(16):
            ps = b_ps.tile([128, 512], F32, name=f"psv_{mt}", tag="psv")
            for k in range(4):
                nc.tensor.matmul(ps[:], kvav[:, k, mt * 128:(mt + 1) * 128],
                                 wvv[:, k, :], start=(k == 0), stop=(k == 3))
            nc.vector.tensor_copy(vv[:, mt, :], ps[:])


def _phase_q(nc, tc, io, agqlv, qt_sb, prefetch_cb):
    """Head-split q^T projection into resident SBUF qt; rope (pe) chunks
    are computed FIRST so the serialized vector rope chain overlaps the
    nope matmuls that follow."""
    qtv = qt_sb[:].rearrange("p (c t) -> p c t", c=6)
    with (
        tc.tile_pool(name="c_qa", bufs=1) as c_qa,
        tc.tile_pool(name="c_tab", bufs=1) as c_tab,
        tc.tile_pool(name="c_w", bufs=6) as c_w,
        tc.tile_pool(name="c_tmp", bufs=2) as c_tmp,
        tc.tile_pool(name="c_ps", bufs=2, space="PSUM") as c_ps,
    ):
        cos_sb = c_tab.tile([128, T], F32R, name="cos_sb")
        sin_sb = c_tab.tile([128, T], F32R, name="sin_sb")
        nc.sync.dma_start(cos_sb[:], io["cosT"][:])
        nc.sync.dma_start(sin_sb[:], io["sinT"][:])
        # all wqb weight chunks DMA'd up-front (before the big qa DMAs) so
        # the sync queue can't head-of-line block the first matmuls
        morder = (4, 5, 0, 1, 2, 3)   # pe chunks first
        wts = {}
        for m in morder:
            wt = c_w.tile([128, 12 * 128], BF16, name=f"cw_{m}", tag="cw")
            nc.sync.dma_start(
                wt[:].rearrange("p (k c) -> p k c", k=12),
                io["wqb"][:].rearrange("(k p) c -> p k c", p=128)[
                    :, :, m * 128:(m + 1) * 128])
            wts[m] = wt
        # gathered q latents: 12 chunks split across the two HW DMA queues
        # (Sync + Activation) so the strided gathers land ~2x faster
        qa_sb = c_qa.tile([128, 12 * T], BF16, name="qa_sb")
        qav = qa_sb[:].rearrange("p (k t) -> p k t", k=12)
        for k in range(12):
            eng = nc.sync if k % 2 == 0 else nc.scalar
            eng.dma_start(
                qav[:, k, :].rearrange("p (r t) -> p r t", r=NCORES),
                agqlv[k * 128:(k + 1) * 128])
        prefetch_cb()
        pestage = c_tab.tile([128, 2 * T], F32R, name="pestage")

        def rope_batch(m, qb):
            st = qtv[:, m, qb * 512:(qb + 1) * 512]
            pe = pestage[:, (m - 4) * T + qb * 512:(m - 4) * T + (qb + 1) * 512]
            cs = cos_sb[:, qb * 512:(qb + 1) * 512]
            sn = sin_sb[:, qb * 512:(qb + 1) * 512]
            for half in range(2):
                r0 = 64 * half
                x1 = pe[r0:r0 + 32, :]
                x2 = pe[r0 + 32:r0 + 64, :]
                t1 = c_tmp.tile([32, 512], F32R,
                                name=f"ct1_{m}_{qb}_{half}", tag="ct1")
                t2 = c_tmp.tile([32, 512], F32R,
                                name=f"ct2_{m}_{qb}_{half}", tag="ct2")
                nc.vector.tensor_tensor(t1[:], x1, cs[r0:r0 + 32, :], op=MUL)
                nc.vector.tensor_tensor(t2[:], x2, sn[r0 + 32:r0 + 64, :],
                                        op=MUL)
                nc.vector.tensor_tensor(st[r0:r0 + 32, :], t1[:], t2[:],
                                        op=SUB)
                nc.vector.tensor_tensor(t1[:], x1, sn[r0:r0 + 32, :], op=MUL)
                nc.vector.tensor_tensor(t2[:], x2, cs[r0 + 32:r0 + 64, :],
                                        op=MUL)
                nc.vector.tensor_tensor(st[r0 + 32:r0 + 64, :], t1[:], t2[:],
                                        op=ADD)

        # rope batches are interleaved between the nope chunks' PSUM copies
        # on the vector FIFO: each batch (~16us) paces one nope chunk's
        # matmuls (~15us) without ever blocking a PSUM hand-off
        rope_work = [(m, qb) for qb in range(4) for m in (4, 5)]
        for m in morder:
            wtv = wts[m][:].rearrange("p (k c) -> p k c", k=12)
            pss = [c_ps.tile([128, 512], F32, name=f"psq_{m}_{qb}", tag=f"psq{qb}")
                   for qb in range(4)]
            for k in range(12):
                for qb in range(4):
                    nc.tensor.matmul(pss[qb][:], wtv[:, k, :],
                                     qav[:, k, qb * 512:(qb + 1) * 512],
                                     start=(k == 0), stop=(k == 11))
            for qb in range(4):
                if m < 4:
                    nc.vector.tensor_copy(qtv[:, m, qb * 512:(qb + 1) * 512],
                                          pss[qb][:])
                else:
                    pe = pestage[:, (m - 4) * T + qb * 512:
                                 (m - 4) * T + (qb + 1) * 512]
                    nc.vector.tensor_copy(pe, pss[qb][:])
            if m < 4 and rope_work:
                rope_batch(*rope_work.pop(0))
                rope_batch(*rope_work.pop(0))
        for mq in rope_work:
            rope_batch(*mq)


def _phase_attn(nc, tc, qt_sb, ag2_ins, ag2_outs, ktv, vv, kpe_sb, consts_t,
                oa0):
    """Causal attention, two heads interleaved per pass; bf16 out -> ag2_in.
    q is read directly from resident SBUF (qt_sb)."""
    ones_c, ones_r, tri_sb = (consts_t["ones_cb"], consts_t["ones_r"],
                              consts_t["tri_b"])
    with (
        tc.tile_pool(name="t_p", bufs=8) as t_p,
        tc.tile_pool(name="t_o", bufs=2) as t_o,
        tc.tile_pool(name="t_ps", bufs=3, space="PSUM") as t_ps,
        tc.tile_pool(name="t_bc", bufs=1, space="PSUM") as t_bc,
        tc.tile_pool(name="t_acc", bufs=1, space="PSUM") as t_acc,
    ):
        for qb in range(4):
            for jp in range(HL // 2):
                js = (2 * jp, 2 * jp + 1)
                qf = {}
                dens, ots = {}, {}
                for s, j in enumerate(js):
                    qfn = qt_sb[:, j * T + qb * 512:j * T + (qb + 1) * 512]
                    pc = (4 + j // 2) * T + qb * 512
                    r0 = 64 * (j % 2)
                    qfp = qt_sb[r0:r0 + 64, pc:pc + 512]
                    qf[j] = (qfn, qfp)
                    dens[j] = t_acc.tile([1, 512], F32, name=f"den_{qb}_{j}",
                                         tag=f"den{s}")
                    ots[j] = t_acc.tile([128, 512], F32, name=f"ot_{qb}_{j}",
                                        tag=f"ot{s}")
                kmax = 4 * qb + 4
                for kk in range(kmax):
                    o = kk - 4 * qb
                    c0 = max(0, o) * 128
                    pts = {}
                    for s, j in enumerate(js):
                        qfn, qfp = qf[j]
                        sT = t_ps.tile([128, 512], F32,
                                       name=f"sT_{qb}_{j}_{kk}", tag="sT")
                        nc.tensor.matmul(sT[:, c0:512],
                                         ktv[:, j, kk * 128:(kk + 1) * 128],
                                         qfn[:, c0:512], start=True, stop=False)
                        r0 = 64 * (j % 2)
                        nc.tensor.matmul(sT[:, c0:512],
                                         kpe_sb[r0:r0 + 64,
                                                kk * 128:(kk + 1) * 128],
                                         qfp[:, c0:512], start=False, stop=True)
                        pT = t_p.tile([128, 512], BF16,
                                      name=f"pT_{qb}_{j}_{kk}", tag="pT")
                        nc.scalar.activation(pT[:, c0:512], sT[:, c0:512],
                                             AF.Exp)
                        if o >= 0:
                            nc.vector.tensor_tensor(pT[:, c0:c0 + 128],
                                                    pT[:, c0:c0 + 128],
                                                    tri_sb[:], op=MUL)
                        pts[j] = pT
                    for j in js:
                        pT = pts[j]
                        nc.tensor.matmul(dens[j][:, c0:512], ones_c,
                                         pT[:, c0:512],
                                         start=(kk == 0), stop=(kk == kmax - 1))
                        nc.tensor.matmul(ots[j][:, c0:512],
                                         vv[:, kk, j * 128:(j + 1) * 128],
                                         pT[:, c0:512],
                                         start=(kk == 0), stop=(kk == kmax - 1))
                for s, j in enumerate(js):
                    den, ot = dens[j], ots[j]
                    rden = t_o.tile([1, 512], F32R, name=f"rden_{qb}_{j}",
                                    tag=f"rden{s}")
                    with nc.allow_low_precision(reason="f32r = fp32 bits"):
                        nc.vector.reciprocal(rden[:], den[:])
                    bcp = t_bc.tile([128, 512], F32, name=f"bcp_{qb}_{j}",
                                    tag="bcp")
                    nc.tensor.matmul(bcp[:], ones_r[:1, :], rden[:],
                                     start=True, stop=True)
                    bcs = t_o.tile([128, 512], F32R, name=f"bcs_{qb}_{j}",
                                   tag=f"bcs{s}")
                    nc.vector.tensor_copy(bcs[:], bcp[:])
                    obf = t_o.tile([128, 512], BF16, name=f"obf_{qb}_{j}",
                                   tag=f"obf{s}")
                    nc.vector.tensor_tensor(obf[:], ots[j][:], bcs[:], op=MUL)
                    nc.sync.dma_start(
                        ag2_ins[qb][j * 128:(j + 1) * 128, :], obf[:])
            nc.gpsimd.collective_compute(
                "AllGather", mybir.AluOpType.bypass,
                ins=[ag2_ins[qb][:]], outs=[ag2_outs[qb][:]],
                replica_groups=[list(range(NCORES))],
            )
            if qb == 1:
                # prefetch phase_out's first input block while attention runs
                # (after qb1's collective: ag2[0] has long finished, so this
                # DMA runs immediately without stalling the queue behind it)
                oav0 = oa0[:].rearrange("p (k t) -> p k t", k=32)
                nc.sync.dma_start(
                    oav0, ag2_outs[0][:].rearrange("(k p) t -> p k t", p=128))


def _phase_out(nc, tc, io, ag2_outs, wov, oa0):
    """D-column-split output projection (bf16); wo preloaded upstream,
    tq=0 input prefetched during attention."""
    with (
        tc.tile_pool(name="o_a", bufs=2) as o_a,
        tc.tile_pool(name="o_st", bufs=3) as o_st,
        tc.tile_pool(name="o_ps", bufs=3, space="PSUM") as o_ps,
    ):
        for tq in range(4):
            if tq == 0:
                oav = oa0[:].rearrange("p (k t) -> p k t", k=32)
            else:
                oa = o_a.tile([128, 32 * 512], BF16, name=f"oa_{tq}", tag="oa")
                oav = oa[:].rearrange("p (k t) -> p k t", k=32)
                nc.sync.dma_start(
                    oav, ag2_outs[tq][:].rearrange("(k p) t -> p k t", p=128))
            for d in range(5):
                ps = o_ps.tile([128, 512], F32, name=f"ops_{tq}_{d}", tag="ops")
                for k in range(32):
                    nc.tensor.matmul(ps[:], wov[:, k, d * 128:(d + 1) * 128],
                                     oav[:, k, :], start=(k == 0), stop=(k == 31))
                st = o_st.tile([128, 512], F32, name=f"ost_{tq}_{d}", tag="ost")
                nc.vector.tensor_copy(st[:], ps[:])
                nc.sync.dma_start(
                    io["outT"][d * 128:(d + 1) * 128,
                               tq * 512:(tq + 1) * 512], st[:])


def _build():
    nc = bass.Bass("TRN2", target_bir_lowering=False, debug=False,
                   num_devices=NCORES)
    io = {
        "hT": nc.dram_tensor("hT", [D, TC], BF16, kind="ExternalInput"),
        "wqa": nc.dram_tensor("wqa", [D, QR], BF16, kind="ExternalInput"),
        "wkva": nc.dram_tensor("wkva", [D, LAT], BF16, kind="ExternalInput"),
        "biask": nc.dram_tensor("biask", [128, 5], F32, kind="ExternalInput"),
        "wqb": nc.dram_tensor("wqb", [QR, 6 * 128], BF16, kind="ExternalInput"),
        "wkvbk": nc.dram_tensor("wkvbk", [KVR, HL * NOPE], BF16,
                                kind="ExternalInput"),
        "wkvbv": nc.dram_tensor("wkvbv", [KVR, HL * VH], BF16,
                                kind="ExternalInput"),
        "wo": nc.dram_tensor("wo", [H * VH, DCOL], BF16, kind="ExternalInput"),
        "cosT": nc.dram_tensor("cosT", [128, T], F32R, kind="ExternalInput"),
        "sinT": nc.dram_tensor("sinT", [128, T], F32R, kind="ExternalInput"),
        "cosA": nc.dram_tensor("cosA", [128, TC], F32R, kind="ExternalInput"),
        "sinA": nc.dram_tensor("sinA", [128, TC], F32R, kind="ExternalInput"),
        "tri": nc.dram_tensor("tri", [128, 128], F32R, kind="ExternalInput"),
        "onesin": nc.dram_tensor("onesin", [128, 128], F32R, kind="ExternalInput"),
        "outT": nc.dram_tensor("outT", [DCOL, T], F32, kind="ExternalOutput"),
    }

    with TileContext(nc) as tc:
        with (
            tc.tile_pool(name="dram", bufs=1, space="DRAM") as dram,
            tc.tile_pool(name="consts", bufs=1) as consts,
        ):
            agkv_in = dram.tile([LAT, TC], BF16, name="agkv_in")
            agkv_out = dram.tile([NCORES * LAT, TC], BF16, addr_space="Shared",
                                 name="agkv_out")
            agql_in = dram.tile([QR, TC], BF16, name="agql_in")
            agql_out = dram.tile([NCORES * QR, TC], BF16, addr_space="Shared",
                                 name="agql_out")
            ag2_ins = [dram.tile([HL * VH, 512], BF16, name=f"ag2_in_{qb}")
                       for qb in range(4)]
            ag2_outs = [dram.tile([H * VH, 512], BF16, addr_space="Shared",
                                  name=f"ag2_out_{qb}") for qb in range(4)]

            consts_t = {}
            ones_sb = consts.tile([128, 128], F32R, name="ones_sb")
            nc.sync.dma_start(ones_sb[:], io["onesin"][:])
            consts_t["ones_c"] = ones_sb[:, 0:1]
            consts_t["ones_r"] = ones_sb
            ones_b = consts.tile([128, 1], BF16, name="ones_b")
            nc.vector.tensor_copy(ones_b[:], ones_sb[:, 0:1])
            consts_t["ones_cb"] = ones_b[:]
            trib = consts.tile([128, 128], BF16, name="trib")
            consts_t["tri_b"] = trib
            for nm, srcn, shp in (("tri_sb", "tri", [128, 128]),
                                  ("cosa_sb", "cosA", [128, TC]),
                                  ("sina_sb", "sinA", [128, TC]),
                                  ):
                consts_t[nm] = consts.tile(shp, F32R, name=nm)
                nc.sync.dma_start(consts_t[nm][:], io[srcn][:])
            consts_t["bias_sb"] = consts.tile([128, 5], F32, name="bias_sb")
            nc.sync.dma_start(consts_t["bias_sb"][:], io["biask"][:])
            nc.vector.tensor_copy(trib[:], consts_t["tri_sb"][:])

            agkvv = agkv_out[:].rearrange("(r a) t -> a r t", a=LAT)
            agqlv = agql_out[:].rearrange("(r a) t -> a r t", a=QR)

            with tc.tile_pool(name="persist", bufs=1) as persist:
                kt_sb = persist.tile([128, HL * T], BF16, name="kt_sb")
                ktv = kt_sb[:].rearrange("p (j t) -> p j t", j=HL)
                v_sb = persist.tile([128, 16 * 512], BF16, name="v_sb")
                vv = v_sb[:].rearrange("p (mt c) -> p mt c", mt=16)
                kpe_sb = persist.tile([128, T], BF16, name="kpe_sb")
                qt_sb = persist.tile([128, 6 * T], BF16, name="qt_sb")

                with (
                    tc.tile_pool(name="b_kva", bufs=1) as b_kva,
                    tc.tile_pool(name="b_w", bufs=1) as b_w,
                ):
                    b_tiles = []

                    def prefetch_b():
                        b_tiles.extend(
                            _phase_b_loads(nc, io, agkvv, kpe_sb, b_kva, b_w))

                    with nc.named_scope("phase_a"):
                        _phase_a(nc, tc, io, consts_t, agkv_in, agkv_out,
                                 agql_in, agql_out)
                    with nc.named_scope("phase_q"):
                        _phase_q(nc, tc, io, agqlv, qt_sb, prefetch_b)
                    with nc.named_scope("phase_b"):
                        _phase_b(nc, tc, ktv, vv, *b_tiles)

                with tc.tile_pool(name="opool", bufs=1) as opool:
                    wo_sb = opool.tile([128, 32 * DCOL], BF16, name="wo_sb")
                    wov = wo_sb[:].rearrange("p (k c) -> p k c", k=32)
                    nc.sync.dma_start(
                        wov, io["wo"][:].rearrange("(k p) c -> p k c", p=128))
                    oa0 = opool.tile([128, 32 * 512], BF16, name="oa0")

                    with nc.named_scope("phase_attn"):
                        _phase_attn(nc, tc, qt_sb, ag2_ins, ag2_outs,
                                    ktv, vv, kpe_sb, consts_t, oa0)

                    with nc.named_scope("phase_out"):
                        _phase_out(nc, tc, io, ag2_outs, wov, oa0)
    return nc


def _get_nc():
    if "nc" not in _cache:
        _cache["nc"] = _build()
    return _cache["nc"]


def _prep(inputs):
    h = np.asarray(inputs["h"], np.float32)
    pos = np.asarray(inputs["position_ids"], np.int32)
    Wq_a = np.asarray(inputs["Wq_a"], np.float32)
    gq = np.asarray(inputs["gq"], np.float32)
    Wq_b = np.asarray(inputs["Wq_b"], np.float32)
    Wkv_a = np.asarray(inputs["Wkv_a"], np.float32)
    bkv_a = np.asarray(inputs["bkv_a"], np.float32)
    gkv = np.asarray(inputs["gkv"], np.float32)
    Wkv_b = np.asarray(inputs["Wkv_b"], np.float32)
    Wo = np.asarray(inputs["Wo"], np.float32)

    dperm = np.concatenate([np.arange(0, ROPE, 2), np.arange(1, ROPE, 2)])
    scale = np.float32(1.0 / math.sqrt(QK))

    hT = np.ascontiguousarray(h.T)                      # [D, T]
    wkva = Wkv_a.copy()
    wkva[:, KVR:] = Wkv_a[:, KVR + dperm]
    bias = bkv_a.copy()
    bias[KVR:] = bkv_a[KVR + dperm]
    bm = np.zeros((5, 128), np.float32)
    bm.reshape(-1)[:LAT] = bias
    biask = np.ascontiguousarray(bm.T)                  # [128, 5]

    wqb_eff = (Wq_b * gq[:, None]) * scale              # [QR, H*QK]
    wkvb_eff = Wkv_b * gkv[:, None]                     # [KVR, H*(NOPE+VH)]

    inv = THETA ** (-np.arange(0, ROPE, 2, dtype=np.float32) / ROPE)
    fr = pos.astype(np.float32)[:, None] * inv[None, :]  # [T, 32]
    cosT = np.ascontiguousarray(np.tile(np.cos(fr).T, (4, 1)))  # [128, T]
    sinT = np.ascontiguousarray(np.tile(np.sin(fr).T, (4, 1)))
    tri = np.triu(np.ones((128, 128), np.float32))
    wqa_b = Wq_a.astype(ml_dtypes.bfloat16)
    wkva_b = wkva.astype(ml_dtypes.bfloat16)

    bf16 = ml_dtypes.bfloat16
    in_maps = []
    for c in range(NCORES):
        heads = list(range(HL * c, HL * (c + 1)))
        qcols = [np.arange(hh * QK, hh * QK + NOPE) for hh in heads]
        for pair in range(2):
            for hh in heads[2 * pair:2 * pair + 2]:
                qcols.append(hh * QK + NOPE + dperm)
        kcols = np.concatenate(
            [np.arange(hh * (NOPE + VH), hh * (NOPE + VH) + NOPE)
             for hh in heads])
        vcols = np.concatenate(
            [np.arange(hh * (NOPE + VH) + NOPE, (hh + 1) * (NOPE + VH))
             for hh in heads])
        in_maps.append({
            "hT": np.ascontiguousarray(hT[:, c * TC:(c + 1) * TC]).astype(bf16),
            "wqa": wqa_b,
            "wkva": wkva_b,
            "biask": biask,
            "wqb": np.ascontiguousarray(wqb_eff[:, np.concatenate(qcols)]).astype(bf16),
            "wkvbk": np.ascontiguousarray(wkvb_eff[:, kcols]).astype(bf16),
            "wkvbv": np.ascontiguousarray(wkvb_eff[:, vcols]).astype(bf16),
            "wo": np.ascontiguousarray(Wo[:, c * DCOL:(c + 1) * DCOL]).astype(bf16),
            "cosT": cosT,
            "sinT": sinT,
            "cosA": np.ascontiguousarray(cosT[:, c * TC:(c + 1) * TC]),
            "sinA": np.ascontiguousarray(sinT[:, c * TC:(c + 1) * TC]),
            "tri": tri,
            "onesin": np.ones((128, 128), np.float32),
        })
    return in_maps


def kernel(**inputs):
    nc = _get_nc()
    in_maps = _prep(inputs)
    res = bass_utils.run_bass_kernel_spmd(
        nc, in_maps, core_ids=list(range(NCORES)), trace=TRACE[0])
    LAST_RESULT[0] = res
    out = np.empty((T, D), np.float32)
    for c in range(NCORES):
        out[:, c * DCOL:(c + 1) * DCOL] = res.results[c]["outT"].T
    return out
